# revision 12
# baseline (speedup 1.0000x reference)
"""CrossAttention kernel for 8 Trainium2 NeuronCores (Bass/Tile).

Sharding: tensor-parallel over heads. Core i handles heads {2i, 2i+1} for
both batch elements. LayerNorm scale/bias and the q-scale are folded into
the projection weights on the host; the per-token LN affine (1/sigma, mu)
is applied on-device (sums via DVE tree-reduction + small PE matmuls,
mu-correction as an extra K=1 contraction row). Scores are computed
transposed [key, q] so the attention-weighted sum over keys maps onto the
PE contraction axis; the softmax denominator rides the AV matmul as a
ones-column of V. alibi ships bf16 and is added to the f32 scores by a
mix of PE identity-matmuls / DVE / Pool(gpsimd) adds (engine balancing).
Host gather: sum the 8 partial [dout, tok] fp16 projections, add bo,
transpose back.
"""

import os
import sys

for _p in ("/opt/trn_rl_repo", "/root/.axon_site/_ro/trn_rl_repo"):
    if os.path.isdir(_p) and _p not in sys.path:
        sys.path.insert(0, _p)

import numpy as np
import ml_dtypes

import concourse.bass as bass
import concourse.tile as tile
from concourse import bacc, mybir
from concourse.masks import make_identity

BF16 = ml_dtypes.bfloat16
F16 = np.float16

HEADS = 16
N_CORES = 8
H_PER_CORE = HEADS // N_CORES  # 2
DH = 64
LN_EPS = 1e-5

B = 2
N_TOK = 2048
D = 1024

QT = 512            # query tile (free dim of scores matmuls)
QP = 1024           # query pair-tile (2 x QT processed per kt visit)
KT = 128            # key tile (partition dim of scoresT)
TT = 512            # token tile for LN/projection phase
N_DT = D // 128     # 8 contraction tiles of 128 over d

# alibi-add engine schedule: how the (b, qp, kt, j) score tiles get their
# alibi added. 'V' = DVE tensor_add, 'P' = PE identity-matmul accumulate,
# 'G' = Pool/gpsimd tensor_add.  Tuned against TimelineSim engine busy%.
ADD_PAT = ("V", "V", "P")


def build_program(n_tok=N_TOK):
    """Build the single-core SPMD Bass program. Returns nc."""
    nc = bacc.Bacc("TRN2")
    f32 = mybir.dt.float32
    f32r = mybir.dt.float32r
    bf16 = mybir.dt.bfloat16
    f16 = mybir.dt.float16

    n_tt = n_tok // TT          # token tiles per batch
    n_qp = n_tok // QP          # query pair-tiles per batch
    n_kt = n_tok // KT          # key tiles per batch

    # ---- DRAM parameters (per-core shards, host-prepped) ----
    xT = nc.declare_dram_parameter("xT", [B, D, n_tok], bf16, isOutput=False)
    cT = nc.declare_dram_parameter("cT", [B, D, n_tok], bf16, isOutput=False)
    alibiT = nc.declare_dram_parameter(
        "alibiT", [H_PER_CORE, n_tok, n_tok], bf16, isOutput=False)
    wqT = nc.declare_dram_parameter("wqT", [D, 128], bf16, isOutput=False)
    wkT = nc.declare_dram_parameter("wkT", [D, 128], bf16, isOutput=False)
    wvT = nc.declare_dram_parameter("wvT", [D, 128], bf16, isOutput=False)
    # rows: -wbar_q, -wbar_k, -wbar_v   (lhsT for the K=1 mu-correction row)
    wbar = nc.declare_dram_parameter("wbar", [3, 128], bf16, isOutput=False)
    woT = nc.declare_dram_parameter("woT", [128, D], bf16, isOutput=False)
    # columns: q/k/v projection bias (ln_b folded through W), fp32
    pbias = nc.declare_dram_parameter("pbias", [128, 3], f32, isOutput=False)

    outT = nc.declare_dram_parameter(
        "outT", [D, B * n_tok], f16, isOutput=True)

    xT_r = xT.rearrange("b (dt p) n -> b p dt n", p=128)
    cT_r = cT.rearrange("b (dt p) n -> b p dt n", p=128)
    woT_r = woT.rearrange("c (dt n) -> c dt n", n=128)
    outT_r = outT.rearrange("(dt p) n -> p dt n", p=128)

    # alibi-add schedule, fixed at trace time
    pat_n = [0]

    def next_add_type():
        t = ADD_PAT[pat_n[0] % len(ADD_PAT)]
        pat_n[0] += 1
        return t

    with tile.TileContext(nc) as tc:
        with tc.tile_pool(name="const", bufs=1) as const_pool:
            ident = const_pool.tile([128, 128], bf16)
            make_identity(nc, ident)
            zeros128 = const_pool.tile([128, 1], f32)
            nc.vector.memset(zeros128, 0.0)
            eps4 = const_pool.tile([4, 1], f32)
            nc.vector.memset(eps4, LN_EPS)
            # stats lhsT: onehot[:, u, j] is all-ones iff j == u
            onehot = const_pool.tile([128, n_tt, 4], bf16)
            nc.vector.memset(onehot, 0.0)
            for u in range(n_tt):
                nc.vector.memset(onehot[:, u, u:u + 1], 1.0)

            wq_sb = const_pool.tile([128, N_DT, 128], bf16)
            wk_sb = const_pool.tile([128, N_DT, 128], bf16)
            wv_sb = const_pool.tile([128, N_DT, 128], bf16)
            nc.sync.dma_start(out=wq_sb, in_=wqT.rearrange("(dt p) c -> p dt c", p=128))
            nc.sync.dma_start(out=wk_sb, in_=wkT.rearrange("(dt p) c -> p dt c", p=128))
            nc.sync.dma_start(out=wv_sb, in_=wvT.rearrange("(dt p) c -> p dt c", p=128))
            wbar_sb = const_pool.tile([1, 3, 128], bf16)
            nc.sync.dma_start(out=wbar_sb, in_=wbar[None, :, :])
            wo_sb = const_pool.tile([128, N_DT, 128], bf16)
            nc.sync.dma_start(out=wo_sb, in_=woT_r)
            pbias_sb = const_pool.tile([128, 3], f32)
            nc.sync.dma_start(out=pbias_sb, in_=pbias[:, :])

            # persistent activations: [c(128), b, tok]
            qT_sb = const_pool.tile([128, B, n_tok], f32r)
            kT_sb = const_pool.tile([128, B, n_tok], f32r)
            vT_sb = const_pool.tile([128, B, n_tok], bf16)
            # v natural (+ones col): [key(128), b*n_kt*h, 66]
            vaug_sb = const_pool.tile([128, B * n_kt * H_PER_CORE, 66], bf16)
            nc.vector.memset(vaug_sb[:, :, 64:65], 1.0)

            def vaug_idx(b, kt, h):
                return (b * n_kt + kt) * H_PER_CORE + h

            # ============ Phase A: LN stats + apply + QKV projections ========
            with tc.tile_pool(name="raw_p", bufs=n_tt + 2) as raw_p, \
                 tc.tile_pool(name="sq_p", bufs=2) as sq_p, \
                 tc.tile_pool(name="tree_p", bufs=6) as tree_p, \
                 tc.tile_pool(name="acc_p", bufs=4) as acc_p, \
                 tc.tile_pool(name="pha", bufs=3) as pha, \
                 tc.tile_pool(name="st_ps", bufs=2, space="PSUM") as st_ps, \
                 tc.tile_pool(name="pj_ps", bufs=3, space="PSUM") as pj_ps, \
                 tc.tile_pool(name="stat_sb", bufs=2) as stat_sb:
                for src_i, src_r in ((0, xT_r), (1, cT_r)):
                    for b in range(B):
                        # --- DVE tree-reduce over dt; PE folds partitions ---
                        sx = st_ps.tile([4, TT], f32, tag="st", name="sx")
                        sxx = st_ps.tile([4, TT], f32, tag="st", name="sxx")
                        raws = []
                        for u in range(n_tt):
                            raw = raw_p.tile([128, N_DT, TT], bf16, tag="raw")
                            raws.append(raw)
                            nc.sync.dma_start(
                                out=raw, in_=src_r[b, :, :, u * TT:(u + 1) * TT])
                            t1 = tree_p.tile([128, 4, TT], bf16, tag="tr", name="t1")
                            nc.vector.tensor_add(
                                t1, raw[:, 0:4, :], raw[:, 4:8, :])
                            t2 = tree_p.tile([128, 2, TT], bf16, tag="tr", name="t2")
                            nc.vector.tensor_add(
                                t2, t1[:, 0:2, :], t1[:, 2:4, :])
                            ax = acc_p.tile([128, TT], bf16, tag="ac", name="ax")
                            nc.vector.tensor_add(ax, t2[:, 0, :], t2[:, 1, :])
                            sq = sq_p.tile([128, N_DT, TT], bf16, tag="sq")
                            nc.vector.tensor_mul(sq, raw, raw)
                            q1 = tree_p.tile([128, 4, TT], bf16, tag="tr", name="q1")
                            nc.vector.tensor_add(
                                q1, sq[:, 0:4, :], sq[:, 4:8, :])
                            q2 = tree_p.tile([128, 2, TT], bf16, tag="tr", name="q2")
                            nc.vector.tensor_add(
                                q2, q1[:, 0:2, :], q1[:, 2:4, :])
                            axx = acc_p.tile([128, TT], bf16, tag="ac", name="axx")
                            nc.vector.tensor_add(axx, q2[:, 0, :], q2[:, 1, :])
                            nc.tensor.matmul(
                                sx, onehot[:, u, :], ax,
                                start=(u == 0), stop=(u == n_tt - 1))
                            nc.tensor.matmul(
                                sxx, onehot[:, u, :], axx,
                                start=(u == 0), stop=(u == n_tt - 1))
                        # --- batched LN math on [n_tt, TT] rows ---
                        e = stat_sb.tile([4, TT], f32, tag="e")
                        nc.vector.tensor_scalar_mul(e, sx, 1.0 / D)
                        ee = stat_sb.tile([4, TT], f32, tag="ee")
                        nc.vector.tensor_mul(ee, e, e)
                        var = stat_sb.tile([4, TT], f32, tag="var")
                        # var*D = Sxx - D*ee
                        nc.vector.scalar_tensor_tensor(
                            out=var, in0=ee, scalar=float(-D), in1=sxx,
                            op0=mybir.AluOpType.mult, op1=mybir.AluOpType.add)
                        lnv = stat_sb.tile([4, TT], f32, tag="lnv")
                        nc.scalar.activation(
                            out=lnv, in_=var, func=mybir.ActivationFunctionType.Ln,
                            bias=eps4[:, 0:1], scale=1.0 / D)
                        invs = stat_sb.tile([4, TT], f32, tag="invs")
                        nc.scalar.activation(
                            out=invs, in_=lnv, func=mybir.ActivationFunctionType.Exp,
                            bias=zeros128[0:4, 0:1], scale=-0.5)
                        invs_bf = stat_sb.tile([4, TT], bf16, tag="invs_bf")
                        nc.vector.tensor_copy(invs_bf, invs)
                        m_bf = stat_sb.tile([4, TT], bf16, tag="m_bf")
                        nc.vector.tensor_mul(m_bf, e, invs)
                        # restage rows at partition 0 (matmul rhs and
                        # partition_broadcast both need base partition 0)
                        m_row = stat_sb.tile([1, n_tt, TT], bf16, tag="m_row")
                        inv_row = stat_sb.tile([1, n_tt, TT], bf16, tag="inv_row")
                        for u in range(n_tt):
                            nc.sync.dma_start(
                                out=m_row[:, u, :], in_=m_bf[u:u + 1, :])
                            nc.sync.dma_start(
                                out=inv_row[:, u, :], in_=invs_bf[u:u + 1, :])

                        # --- apply 1/sigma in place: raw <- raw * isb ---
                        for u in range(n_tt):
                            isb = pha.tile([128, TT], bf16, tag="isb")
                            nc.gpsimd.partition_broadcast(
                                isb, inv_row[:, u, :])
                            for dt in range(N_DT):
                                nc.vector.tensor_mul(
                                    raws[u][:, dt, :], raws[u][:, dt, :], isb)
                        # --- projections, u-pairs batched in PSUM, dt-outer
                        # for lhsT (Ldweights) reuse across the u tiles ---
                        if src_i == 0:
                            projs = ((0, wq_sb, qT_sb),)
                        else:
                            projs = ((1, wk_sb, kT_sb), (2, wv_sb, vT_sb))
                        half = n_tt // 2
                        for wi, w_sb, dst in projs:
                            pss = [pj_ps.tile([128, half, TT], f32,
                                              tag="ps", name=f"ps{pi}")
                                   for pi in range(2)]
                            for dt in range(N_DT):
                                for u in range(n_tt):
                                    nc.tensor.matmul(
                                        pss[u // half][:, u % half, :],
                                        w_sb[:, dt, :], raws[u][:, dt, :],
                                        start=(dt == 0), stop=False)
                            for pi, ps in enumerate(pss):
                                for hi in range(half):
                                    nc.tensor.matmul(
                                        ps[:, hi, :], wbar_sb[:, wi, :],
                                        m_row[:, pi * half + hi, :],
                                        start=False, stop=True)
                            hw = half * TT
                            for pi, ps in enumerate(pss):
                                nc.scalar.activation(
                                    out=dst[:, b, pi * hw:(pi + 1) * hw],
                                    in_=ps,
                                    func=mybir.ActivationFunctionType.Identity,
                                    bias=pbias_sb[:, wi:wi + 1], scale=1.0)
                        # --- v natural (transpose vT) once per ctx batch ---
                        if src_i == 1:
                            for kt in range(n_kt):
                                vt = st_ps.tile([128, 128], bf16, tag="st", name="vt")
                                nc.tensor.transpose(
                                    vt, vT_sb[:, b, kt * KT:(kt + 1) * KT], ident)
                                for h in range(H_PER_CORE):
                                    nc.vector.tensor_copy(
                                        vaug_sb[:, vaug_idx(b, kt, h), 0:64],
                                        vt[:, h * 64:(h + 1) * 64])

            # ============ Phase B: attention + output projection =============
            with tc.tile_pool(name="alq", bufs=n_kt + 2) as alq, \
                 tc.tile_pool(name="ex_p", bufs=3) as ex_p, \
                 tc.tile_pool(name="scs_p", bufs=3) as scs_p, \
                 tc.tile_pool(name="phb2", bufs=2) as phb2, \
                 tc.tile_pool(name="fo_p", bufs=3) as fo_p, \
                 tc.tile_pool(name="sc_ps", bufs=2, space="PSUM") as sc_ps, \
                 tc.tile_pool(name="av_ps", bufs=2, space="PSUM") as av_ps:
                for qp in range(n_qp):
                    q0 = qp * QP
                    # alibi tiles for this qp: loaded once, used by both b
                    al_tiles = []
                    for kt in range(n_kt):
                        al = alq.tile([128, H_PER_CORE, QP], bf16, tag="al")
                        nc.sync.dma_start(
                            out=al,
                            in_=alibiT[:, kt * KT:(kt + 1) * KT,
                                       q0:q0 + QP].rearrange("h p n -> p h n"))
                        al_tiles.append(al)
                    for b in range(B):
                        av = [av_ps.tile([65, 2, QT], f32, tag="av",
                                         name=f"av{h}")
                              for h in range(H_PER_CORE)]
                        for kt in range(n_kt):
                            k_sl = slice(kt * KT, (kt + 1) * KT)
                            al = al_tiles[kt]
                            typs = [next_add_type() for _ in range(2)]
                            scs = [sc_ps.tile([128, H_PER_CORE, QT], f32,
                                              tag="sc", name=f"sc{j}")
                                   for j in range(2)]
                            # scores: h-outer for lhsT reuse across j
                            for h in range(H_PER_CORE):
                                c_sl = slice(h * 64, (h + 1) * 64)
                                for j in range(2):
                                    qs = slice(q0 + j * QT, q0 + (j + 1) * QT)
                                    nc.tensor.matmul(
                                        scs[j][:, h, :],
                                        kT_sb[c_sl, b, k_sl],
                                        qT_sb[c_sl, b, qs],
                                        start=True,
                                        stop=(typs[j] != "P"),
                                        tile_position=(h * 64, 0))
                            ex = ex_p.tile([128, 2, H_PER_CORE, QT], bf16,
                                           tag="ex")
                            for j in range(2):
                                j_sl = slice(j * QT, (j + 1) * QT)
                                if typs[j] == "P":
                                    for h in range(H_PER_CORE):
                                        nc.tensor.matmul(
                                            scs[j][:, h, :], ident,
                                            al[:, h, j_sl],
                                            start=False, stop=True)
                                    exp_in = scs[j]
                                else:
                                    eng = (nc.vector if typs[j] == "V"
                                           else nc.gpsimd)
                                    ssb = scs_p.tile(
                                        [128, H_PER_CORE, QT], f32, tag="ssb")
                                    eng.tensor_add(ssb, scs[j], al[:, :, j_sl])
                                    exp_in = ssb
                                nc.scalar.activation(
                                    out=ex[:, j, :, :], in_=exp_in,
                                    func=mybir.ActivationFunctionType.Exp,
                                    bias=zeros128[:, 0:1], scale=1.0)
                            for h in range(H_PER_CORE):
                                for j in range(2):
                                    nc.tensor.matmul(
                                        av[h][:, j, :],
                                        vaug_sb[:, vaug_idx(b, kt, h), 0:65],
                                        ex[:, j, h, :],
                                        start=(kt == 0),
                                        stop=(kt == n_kt - 1))
                        # normalize: rows 0:64 outT_h, row 64 the denominator
                        den = phb2.tile([1, 4, QT], f32, tag="dn", name="den")
                        for h in range(H_PER_CORE):
                            nc.vector.tensor_copy(den[:, 2 * h:2 * h + 2, :],
                                                  av[h][64:65, :, :])
                        rden = phb2.tile([1, 4, QT], f32, tag="dn", name="rden")
                        nc.vector.reciprocal_approx_fast(rden, den)
                        o_sb = phb2.tile([128, 2, QT], bf16, tag="o_sb")
                        for h in range(H_PER_CORE):
                            rb = phb2.tile([64, 2, QT], f32, tag="rb", name=f"rb{h}")
                            nc.gpsimd.partition_broadcast(
                                rb, rden[:, 2 * h:2 * h + 2, :])
                            nc.vector.tensor_mul(
                                o_sb[h * 64:(h + 1) * 64, :, :],
                                av[h][0:64, :, :], rb)
                        # output projection: [dout, q] partials, fp16 out
                        for dp in range(N_DT // 2):
                            fo = fo_p.tile([128, 2, 2, QT], f16, tag="fo")
                            for di in range(2):
                                dt = dp * 2 + di
                                fp = av_ps.tile([128, H_PER_CORE, QT], f32,
                                                tag="av", name="fp")
                                for j in range(2):
                                    nc.tensor.matmul(
                                        fp[:, j, :], wo_sb[:, dt, :],
                                        o_sb[:, j, :], start=True,
                                        stop=True)
                                nc.scalar.activation(
                                    out=fo[:, di, :, :], in_=fp,
                                    func=mybir.ActivationFunctionType.Identity,
                                    bias=zeros128[:, 0:1], scale=1.0)
                            nc.sync.dma_start(
                                out=outT_r[:, dp * 2:dp * 2 + 2,
                                           b * n_tok + q0:
                                           b * n_tok + q0 + QP],
                                in_=fo)
    nc.compile()
    return nc


_NC_CACHE = {}


def _get_program(n_tok=N_TOK):
    if n_tok not in _NC_CACHE:
        _NC_CACHE[n_tok] = build_program(n_tok)
    return _NC_CACHE[n_tok]


def _prep_in_maps(x, context, alibi, Wq, Wk, Wv, Wo, bo, ln_w, ln_b):
    b, n, d = x.shape
    scale = (d // HEADS) ** -0.5

    x = np.asarray(x, dtype=np.float32)
    context = np.asarray(context, dtype=np.float32)
    alibi = np.asarray(alibi, dtype=np.float32)
    Wq, Wk, Wv, Wo = (np.asarray(w, dtype=np.float32) for w in (Wq, Wk, Wv, Wo))
    ln_w = np.asarray(ln_w, dtype=np.float32)
    ln_b = np.asarray(ln_b, dtype=np.float32)

    xT = np.ascontiguousarray(x.transpose(0, 2, 1)).astype(BF16)
    cT = np.ascontiguousarray(context.transpose(0, 2, 1)).astype(BF16)

    in_maps = []
    for ci in range(N_CORES):
        h0 = ci * H_PER_CORE
        cs = slice(h0 * DH, (h0 + H_PER_CORE) * DH)  # this core's 128 channels
        alT = np.ascontiguousarray(
            alibi[0, h0:h0 + H_PER_CORE].transpose(0, 2, 1)).astype(BF16)

        wq_s = (Wq[cs] * ln_w[None, :]) * scale          # [128, d]
        wk_s = Wk[cs] * ln_w[None, :]
        wv_s = Wv[cs] * ln_w[None, :]
        wbar = np.stack([
            -wq_s.sum(axis=1), -wk_s.sum(axis=1), -wv_s.sum(axis=1)])
        pb = np.stack([
            (Wq[cs] @ ln_b) * scale, Wk[cs] @ ln_b, Wv[cs] @ ln_b]).T  # [128,3]

        in_maps.append({
            "xT": xT,
            "cT": cT,
            "alibiT": alT,
            "wqT": np.ascontiguousarray(wq_s.T).astype(BF16),
            "wkT": np.ascontiguousarray(wk_s.T).astype(BF16),
            "wvT": np.ascontiguousarray(wv_s.T).astype(BF16),
            "wbar": wbar.astype(BF16),
            "woT": np.ascontiguousarray(Wo[:, cs].T).astype(BF16),
            "pbias": np.ascontiguousarray(pb).astype(np.float32),
        })
    return in_maps


def _gather(results, b, n, d, bo):
    acc = np.zeros((d, b * n), dtype=np.float32)
    for r in results:
        acc += r["outT"].astype(np.float32)
    out = acc.reshape(d, b, n).transpose(1, 2, 0)
    out = out + np.asarray(bo, dtype=np.float32)[None, None, :]
    return np.ascontiguousarray(out).astype(np.float32)


def kernel(**inputs):
    from concourse.bass_utils import run_bass_kernel_spmd
    x = inputs["x"]
    b, n, d = x.shape
    nc = _get_program(n)
    in_maps = _prep_in_maps(**inputs)
    res = run_bass_kernel_spmd(nc, in_maps, list(range(N_CORES)))
    return _gather(res.results, b, n, d, inputs["bo"])


def run_profiled(inputs, trace=True):
    from concourse.bass_utils import run_bass_kernel_spmd
    x = inputs["x"]
    b, n, d = x.shape
    nc = _get_program(n)
    in_maps = _prep_in_maps(**inputs)
    res = run_bass_kernel_spmd(nc, in_maps, list(range(N_CORES)), trace=trace)
    return _gather(res.results, b, n, d, inputs["bo"]), res


# revision 14
# speedup vs baseline: 1.2383x; 1.2383x over previous
"""CrossAttention kernel for 8 Trainium2 NeuronCores (Bass/Tile).

Sharding: tensor-parallel over heads. Core i handles heads {2i, 2i+1} for
both batch elements.

Phase A: LayerNorm + QKV projections. ln_w and the q-scale fold into the
projection weights on the host. Per-token sums ride a DVE add-tree
(Sigma_x) and Act-engine Square + PE onehot-matmuls (Sigma_x2); the
projections accumulate W@(x-mu) in PSUM via a K=1 mu-correction row
(plus a pb (x) sigma row for the ln_b-induced bias), and a single DVE op
applies the 1/sigma column scale post-matmul.

Phase B: scores are computed transposed [key, q] so the attention-
weighted sum over keys maps onto the PE contraction axis; the softmax
denominator rides the AV matmul as a ones-column of V. alibi enters as
host-precomputed exp(alibi) in bf16: attn = exp(s) * ealibi, a cheap
all-bf16 DVE multiply (2x mode) applied in place on the exp output.
ealibi tiles are SBUF-resident per query-pair and reused across both
batch elements. Host gather: sum the 8 partial [dout, tok] fp16
projections, add bo, transpose back.
"""

import os
import sys

for _p in ("/opt/trn_rl_repo", "/root/.axon_site/_ro/trn_rl_repo"):
    if os.path.isdir(_p) and _p not in sys.path:
        sys.path.insert(0, _p)

import numpy as np
import ml_dtypes

import concourse.bass as bass
import concourse.tile as tile
from concourse import bacc, mybir
from concourse.masks import make_identity

BF16 = ml_dtypes.bfloat16
F16 = np.float16

HEADS = 16
N_CORES = 8
H_PER_CORE = HEADS // N_CORES  # 2
DH = 64
LN_EPS = 1e-5

B = 2
N_TOK = 2048
D = 1024

QT = 512            # query tile (free dim of scores matmuls)
QP = 1024           # query pair-tile (2 x QT processed per kt visit)
KT = 128            # key tile (partition dim of scoresT)
TT = 512            # token tile for LN/projection phase
N_DT = D // 128     # 8 contraction tiles of 128 over d


def build_program(n_tok=N_TOK):
    """Build the single-core SPMD Bass program. Returns nc."""
    nc = bacc.Bacc("TRN2")
    f32 = mybir.dt.float32
    f32r = mybir.dt.float32r
    bf16 = mybir.dt.bfloat16
    f16 = mybir.dt.float16

    n_tt = n_tok // TT          # token tiles per batch
    n_qp = n_tok // QP          # query pair-tiles per batch
    n_kt = n_tok // KT          # key tiles per batch

    # ---- DRAM parameters (per-core shards, host-prepped) ----
    xT = nc.declare_dram_parameter("xT", [B, D, n_tok], bf16, isOutput=False)
    cT = nc.declare_dram_parameter("cT", [B, D, n_tok], bf16, isOutput=False)
    # host-precomputed exp(alibi), transposed [h, key, q]
    eaT = nc.declare_dram_parameter(
        "eaT", [H_PER_CORE, n_tok, n_tok], bf16, isOutput=False)
    wqT = nc.declare_dram_parameter("wqT", [D, 128], bf16, isOutput=False)
    wkT = nc.declare_dram_parameter("wkT", [D, 128], bf16, isOutput=False)
    wvT = nc.declare_dram_parameter("wvT", [D, 128], bf16, isOutput=False)
    # rows: -wbar_q, -wbar_k, -wbar_v   (lhsT for the K=1 mu-correction row)
    wbar = nc.declare_dram_parameter("wbar", [3, 128], bf16, isOutput=False)
    woT = nc.declare_dram_parameter("woT", [128, D], bf16, isOutput=False)
    # rows: q/k/v projection bias (ln_b folded through W), bf16 lhsT for
    # the pb (x) sigma rank-1 row
    pbias = nc.declare_dram_parameter("pbias", [3, 128], bf16, isOutput=False)

    outT = nc.declare_dram_parameter(
        "outT", [D, B * n_tok], f16, isOutput=True)

    xT_r = xT.rearrange("b (dt p) n -> b p dt n", p=128)
    cT_r = cT.rearrange("b (dt p) n -> b p dt n", p=128)
    woT_r = woT.rearrange("c (dt n) -> c dt n", n=128)
    outT_r = outT.rearrange("(dt p) n -> p dt n", p=128)

    with tile.TileContext(nc) as tc:
        with tc.tile_pool(name="const", bufs=1) as const_pool:
            ident = const_pool.tile([128, 128], bf16)
            make_identity(nc, ident)
            zeros128 = const_pool.tile([128, 1], f32)
            nc.vector.memset(zeros128, 0.0)
            eps4 = const_pool.tile([4, 1], f32)
            nc.vector.memset(eps4, LN_EPS)
            # stats lhsT: onehot[:, u, j] is all-ones iff j == u
            onehot = const_pool.tile([128, n_tt, 4], bf16)
            nc.vector.memset(onehot, 0.0)
            for u in range(n_tt):
                nc.vector.memset(onehot[:, u, u:u + 1], 1.0)

            wq_sb = const_pool.tile([128, N_DT, 128], bf16)
            wk_sb = const_pool.tile([128, N_DT, 128], bf16)
            wv_sb = const_pool.tile([128, N_DT, 128], bf16)
            nc.sync.dma_start(out=wq_sb, in_=wqT.rearrange("(dt p) c -> p dt c", p=128))
            nc.sync.dma_start(out=wk_sb, in_=wkT.rearrange("(dt p) c -> p dt c", p=128))
            nc.sync.dma_start(out=wv_sb, in_=wvT.rearrange("(dt p) c -> p dt c", p=128))
            wbar_sb = const_pool.tile([1, 3, 128], bf16)
            nc.sync.dma_start(out=wbar_sb, in_=wbar[None, :, :])
            wo_sb = const_pool.tile([128, N_DT, 128], bf16)
            nc.sync.dma_start(out=wo_sb, in_=woT_r)
            pbias_sb = const_pool.tile([1, 3, 128], bf16)
            nc.sync.dma_start(out=pbias_sb, in_=pbias[None, :, :])

            # persistent activations: [c(128), b, tok]
            qT_sb = const_pool.tile([128, B, n_tok], f32r)
            kT_sb = const_pool.tile([128, B, n_tok], f32r)
            vT_sb = const_pool.tile([128, B, n_tok], bf16)
            # v natural (+ones col): [key(128), b*n_kt*h, 66]
            vaug_sb = const_pool.tile([128, B * n_kt * H_PER_CORE, 66], bf16)
            nc.vector.memset(vaug_sb[:, :, 64:65], 1.0)

            def vaug_idx(b, kt, h):
                return (b * n_kt + kt) * H_PER_CORE + h

            # ============ Phase A: LN stats + QKV projections ========
            with tc.tile_pool(name="raw_p", bufs=n_tt + 2) as raw_p, \
                 tc.tile_pool(name="sq_p", bufs=2) as sq_p, \
                 tc.tile_pool(name="tree_p", bufs=4) as tree_p, \
                 tc.tile_pool(name="acc_p", bufs=3) as acc_p, \
                 tc.tile_pool(name="isb_p", bufs=2) as isb_p, \
                 tc.tile_pool(name="st_ps", bufs=2, space="PSUM") as st_ps, \
                 tc.tile_pool(name="pj_ps", bufs=3, space="PSUM") as pj_ps, \
                 tc.tile_pool(name="stat_sb", bufs=2) as stat_sb:
                for src_i, src_r in ((0, xT_r), (1, cT_r)):
                    for b in range(B):
                        # --- Sigma_x: DVE tree; Sigma_x2: Act square + PE ---
                        sx = st_ps.tile([4, TT], f32, tag="st", name="sx")
                        sxx = st_ps.tile([4, TT], f32, tag="st", name="sxx")
                        raws = []
                        for u in range(n_tt):
                            raw = raw_p.tile([128, N_DT, TT], bf16, tag="raw")
                            raws.append(raw)
                            nc.sync.dma_start(
                                out=raw, in_=src_r[b, :, :, u * TT:(u + 1) * TT])
                            t1 = tree_p.tile([128, 4, TT], bf16, tag="tr", name="t1")
                            nc.vector.tensor_add(
                                t1, raw[:, 0:4, :], raw[:, 4:8, :])
                            t2 = tree_p.tile([128, 2, TT], bf16, tag="tr", name="t2")
                            nc.vector.tensor_add(
                                t2, t1[:, 0:2, :], t1[:, 2:4, :])
                            ax = acc_p.tile([128, TT], bf16, tag="ac", name="ax")
                            nc.vector.tensor_add(ax, t2[:, 0, :], t2[:, 1, :])
                            sq = sq_p.tile([128, N_DT, TT], bf16, tag="sq")
                            nc.scalar.activation(
                                out=sq, in_=raw,
                                func=mybir.ActivationFunctionType.Square,
                                bias=zeros128[:, 0:1], scale=1.0)
                            nc.tensor.matmul(
                                sx, onehot[:, u, :], ax,
                                start=(u == 0), stop=(u == n_tt - 1))
                            for dt in range(N_DT):
                                nc.tensor.matmul(
                                    sxx, onehot[:, u, :], sq[:, dt, :],
                                    start=(u == 0 and dt == 0),
                                    stop=(u == n_tt - 1 and dt == N_DT - 1))
                        # --- batched LN math on [n_tt, TT] rows ---
                        e = stat_sb.tile([4, TT], f32, tag="e")
                        nc.vector.tensor_scalar_mul(e, sx, 1.0 / D)
                        ee = stat_sb.tile([4, TT], f32, tag="ee")
                        nc.vector.tensor_mul(ee, e, e)
                        var = stat_sb.tile([4, TT], f32, tag="var")
                        # var*D = Sxx - D*ee
                        nc.vector.scalar_tensor_tensor(
                            out=var, in0=ee, scalar=float(-D), in1=sxx,
                            op0=mybir.AluOpType.mult, op1=mybir.AluOpType.add)
                        lnv = stat_sb.tile([4, TT], f32, tag="lnv")
                        nc.scalar.activation(
                            out=lnv, in_=var, func=mybir.ActivationFunctionType.Ln,
                            bias=eps4[:, 0:1], scale=1.0 / D)
                        invs = stat_sb.tile([4, TT], f32, tag="invs")
                        nc.scalar.activation(
                            out=invs, in_=lnv, func=mybir.ActivationFunctionType.Exp,
                            bias=zeros128[0:4, 0:1], scale=-0.5)
                        sig = stat_sb.tile([4, TT], bf16, tag="sig")
                        nc.scalar.activation(
                            out=sig, in_=var,
                            func=mybir.ActivationFunctionType.Sqrt,
                            bias=eps4[:, 0:1], scale=1.0 / D)
                        invs_bf = stat_sb.tile([4, TT], bf16, tag="invs_bf")
                        nc.vector.tensor_copy(invs_bf, invs)
                        mu_bf = stat_sb.tile([4, TT], bf16, tag="mu_bf")
                        nc.vector.tensor_copy(mu_bf, e)
                        # restage rows at partition 0 (matmul rhs and
                        # partition_broadcast both need base partition 0)
                        m_row = stat_sb.tile([1, n_tt, TT], bf16, tag="m_row")
                        s_row = stat_sb.tile([1, n_tt, TT], bf16, tag="s_row")
                        inv_row = stat_sb.tile([1, n_tt, TT], bf16, tag="inv_row")
                        for u in range(n_tt):
                            nc.sync.dma_start(
                                out=m_row[:, u, :], in_=mu_bf[u:u + 1, :])
                            nc.sync.dma_start(
                                out=s_row[:, u, :], in_=sig[u:u + 1, :])
                            nc.sync.dma_start(
                                out=inv_row[:, u, :], in_=invs_bf[u:u + 1, :])
                        # broadcast 1/sigma to all partitions, per u
                        isb_all = isb_p.tile([128, n_tt, TT], bf16, tag="isb")
                        for u in range(n_tt):
                            nc.gpsimd.partition_broadcast(
                                isb_all[:, u, :], inv_row[:, u, :])

                        # --- projections: PSUM = W@(x-mu) + pb(x)sigma,
                        # then one DVE column-scale by 1/sigma ---
                        if src_i == 0:
                            projs = ((0, wq_sb, qT_sb, f32),)
                        else:
                            projs = ((1, wk_sb, kT_sb, f32),
                                     (2, wv_sb, vT_sb, bf16))
                        half = n_tt // 2
                        for wi, w_sb, dst, odt in projs:
                            pss = [pj_ps.tile([128, half, TT], f32,
                                              tag="ps", name=f"ps{pi}")
                                   for pi in range(2)]
                            for dt in range(N_DT):
                                for u in range(n_tt):
                                    nc.tensor.matmul(
                                        pss[u // half][:, u % half, :],
                                        w_sb[:, dt, :], raws[u][:, dt, :],
                                        start=(dt == 0), stop=False)
                            for pi, ps in enumerate(pss):
                                for hi in range(half):
                                    u = pi * half + hi
                                    nc.tensor.matmul(
                                        ps[:, hi, :], wbar_sb[:, wi, :],
                                        m_row[:, u, :],
                                        start=False, stop=False)
                                    nc.tensor.matmul(
                                        ps[:, hi, :], pbias_sb[:, wi, :],
                                        s_row[:, u, :],
                                        start=False, stop=True)
                            hw = half * TT
                            for pi, ps in enumerate(pss):
                                out_ap = dst[:, b, pi * hw:(pi + 1) * hw]
                                out_ap = out_ap.rearrange(
                                    "p (h t) -> p h t", h=half)
                                nc.vector.tensor_mul(
                                    out_ap, ps,
                                    isb_all[:, pi * half:(pi + 1) * half, :])
                        # --- v natural (transpose vT) once per ctx batch ---
                        if src_i == 1:
                            for kt in range(n_kt):
                                vt = st_ps.tile([128, 128], bf16, tag="st",
                                                name="vt")
                                nc.tensor.transpose(
                                    vt, vT_sb[:, b, kt * KT:(kt + 1) * KT], ident)
                                for h in range(H_PER_CORE):
                                    nc.vector.tensor_copy(
                                        vaug_sb[:, vaug_idx(b, kt, h), 0:64],
                                        vt[:, h * 64:(h + 1) * 64])

            # ============ Phase B: attention + output projection =============
            with tc.tile_pool(name="alq", bufs=n_kt + 2) as alq, \
                 tc.tile_pool(name="ex_p", bufs=4) as ex_p, \
                 tc.tile_pool(name="phb2", bufs=2) as phb2, \
                 tc.tile_pool(name="fo_p", bufs=3) as fo_p, \
                 tc.tile_pool(name="sc_ps", bufs=2, space="PSUM") as sc_ps, \
                 tc.tile_pool(name="av_ps", bufs=2, space="PSUM") as av_ps:
                for qp in range(n_qp):
                    q0 = qp * QP
                    # exp(alibi) tiles for this qp: loaded once, used by both b
                    ea_tiles = []
                    for kt in range(n_kt):
                        ea = alq.tile([128, H_PER_CORE, QP], bf16, tag="ea")
                        nc.sync.dma_start(
                            out=ea,
                            in_=eaT[:, kt * KT:(kt + 1) * KT,
                                    q0:q0 + QP].rearrange("h p n -> p h n"))
                        ea_tiles.append(ea)
                    for b in range(B):
                        av = [av_ps.tile([65, 2, QT], f32, tag="av",
                                         name=f"av{h}")
                              for h in range(H_PER_CORE)]
                        for kt in range(n_kt):
                            k_sl = slice(kt * KT, (kt + 1) * KT)
                            ea = ea_tiles[kt]
                            scs = [sc_ps.tile([128, H_PER_CORE, QT], f32,
                                              tag="sc", name=f"sc{j}")
                                   for j in range(2)]
                            # scores: h-outer for lhsT reuse across j
                            for h in range(H_PER_CORE):
                                c_sl = slice(h * 64, (h + 1) * 64)
                                for j in range(2):
                                    qs = slice(q0 + j * QT, q0 + (j + 1) * QT)
                                    nc.tensor.matmul(
                                        scs[j][:, h, :],
                                        kT_sb[c_sl, b, k_sl],
                                        qT_sb[c_sl, b, qs],
                                        start=True, stop=True,
                                        tile_position=(h * 64, 0))
                            ex = ex_p.tile([128, 2, H_PER_CORE, QT], bf16,
                                           tag="ex")
                            for j in range(2):
                                nc.scalar.activation(
                                    out=ex[:, j, :, :], in_=scs[j],
                                    func=mybir.ActivationFunctionType.Exp,
                                    bias=zeros128[:, 0:1], scale=1.0)
                                # attn = exp(s) * exp(alibi), in place (bf16)
                                j_sl = slice(j * QT, (j + 1) * QT)
                                nc.vector.tensor_mul(
                                    ex[:, j, :, :], ex[:, j, :, :],
                                    ea[:, :, j_sl])
                            for h in range(H_PER_CORE):
                                for j in range(2):
                                    nc.tensor.matmul(
                                        av[h][:, j, :],
                                        vaug_sb[:, vaug_idx(b, kt, h), 0:65],
                                        ex[:, j, h, :],
                                        start=(kt == 0),
                                        stop=(kt == n_kt - 1))
                        # normalize: rows 0:64 outT_h, row 64 the denominator
                        den = phb2.tile([1, 4, QT], f32, tag="dn", name="den")
                        for h in range(H_PER_CORE):
                            nc.vector.tensor_copy(den[:, 2 * h:2 * h + 2, :],
                                                  av[h][64:65, :, :])
                        rden = phb2.tile([1, 4, QT], f32, tag="dn", name="rden")
                        nc.vector.reciprocal_approx_fast(rden, den)
                        o_sb = phb2.tile([128, 2, QT], bf16, tag="o_sb")
                        for h in range(H_PER_CORE):
                            rb = phb2.tile([64, 2, QT], f32, tag="rb",
                                           name=f"rb{h}")
                            nc.gpsimd.partition_broadcast(
                                rb, rden[:, 2 * h:2 * h + 2, :])
                            nc.vector.tensor_mul(
                                o_sb[h * 64:(h + 1) * 64, :, :],
                                av[h][0:64, :, :], rb)
                        # output projection: [dout, q] partials, fp16 out
                        for dp in range(N_DT // 2):
                            fo = fo_p.tile([128, 2, 2, QT], f16, tag="fo")
                            for di in range(2):
                                dt = dp * 2 + di
                                fp = av_ps.tile([128, H_PER_CORE, QT], f32,
                                                tag="av", name="fp")
                                for j in range(2):
                                    nc.tensor.matmul(
                                        fp[:, j, :], wo_sb[:, dt, :],
                                        o_sb[:, j, :], start=True,
                                        stop=True)
                                nc.vector.tensor_copy(fo[:, di, :, :], fp)
                            nc.sync.dma_start(
                                out=outT_r[:, dp * 2:dp * 2 + 2,
                                           b * n_tok + q0:
                                           b * n_tok + q0 + QP],
                                in_=fo)
    nc.compile()
    return nc


_NC_CACHE = {}


def _get_program(n_tok=N_TOK):
    if n_tok not in _NC_CACHE:
        _NC_CACHE[n_tok] = build_program(n_tok)
    return _NC_CACHE[n_tok]


def _prep_in_maps(x, context, alibi, Wq, Wk, Wv, Wo, bo, ln_w, ln_b):
    b, n, d = x.shape
    scale = (d // HEADS) ** -0.5

    x = np.asarray(x, dtype=np.float32)
    context = np.asarray(context, dtype=np.float32)
    alibi = np.asarray(alibi, dtype=np.float32)
    Wq, Wk, Wv, Wo = (np.asarray(w, dtype=np.float32) for w in (Wq, Wk, Wv, Wo))
    ln_w = np.asarray(ln_w, dtype=np.float32)
    ln_b = np.asarray(ln_b, dtype=np.float32)

    xT = np.ascontiguousarray(x.transpose(0, 2, 1)).astype(BF16)
    cT = np.ascontiguousarray(context.transpose(0, 2, 1)).astype(BF16)
    ealibi = np.exp(alibi)

    in_maps = []
    for ci in range(N_CORES):
        h0 = ci * H_PER_CORE
        cs = slice(h0 * DH, (h0 + H_PER_CORE) * DH)  # this core's 128 channels
        eaTc = np.ascontiguousarray(
            ealibi[0, h0:h0 + H_PER_CORE].transpose(0, 2, 1)).astype(BF16)

        wq_s = (Wq[cs] * ln_w[None, :]) * scale          # [128, d]
        wk_s = Wk[cs] * ln_w[None, :]
        wv_s = Wv[cs] * ln_w[None, :]
        wbar = np.stack([
            -wq_s.sum(axis=1), -wk_s.sum(axis=1), -wv_s.sum(axis=1)])
        pb = np.stack([
            (Wq[cs] @ ln_b) * scale, Wk[cs] @ ln_b, Wv[cs] @ ln_b])  # [3,128]

        in_maps.append({
            "xT": xT,
            "cT": cT,
            "eaT": eaTc,
            "wqT": np.ascontiguousarray(wq_s.T).astype(BF16),
            "wkT": np.ascontiguousarray(wk_s.T).astype(BF16),
            "wvT": np.ascontiguousarray(wv_s.T).astype(BF16),
            "wbar": wbar.astype(BF16),
            "woT": np.ascontiguousarray(Wo[:, cs].T).astype(BF16),
            "pbias": np.ascontiguousarray(pb).astype(BF16),
        })
    return in_maps


def _gather(results, b, n, d, bo):
    acc = np.zeros((d, b * n), dtype=np.float32)
    for r in results:
        acc += r["outT"].astype(np.float32)
    out = acc.reshape(d, b, n).transpose(1, 2, 0)
    out = out + np.asarray(bo, dtype=np.float32)[None, None, :]
    return np.ascontiguousarray(out).astype(np.float32)


def kernel(**inputs):
    from concourse.bass_utils import run_bass_kernel_spmd
    x = inputs["x"]
    b, n, d = x.shape
    nc = _get_program(n)
    in_maps = _prep_in_maps(**inputs)
    res = run_bass_kernel_spmd(nc, in_maps, list(range(N_CORES)))
    return _gather(res.results, b, n, d, inputs["bo"])


def run_profiled(inputs, trace=True):
    from concourse.bass_utils import run_bass_kernel_spmd
    x = inputs["x"]
    b, n, d = x.shape
    nc = _get_program(n)
    in_maps = _prep_in_maps(**inputs)
    res = run_bass_kernel_spmd(nc, in_maps, list(range(N_CORES)), trace=trace)
    return _gather(res.results, b, n, d, inputs["bo"]), res
